# revision 75
# baseline (speedup 1.0000x reference)
"""Trainium2 Bass kernel for nn_DA_conv (dense_cnn) — v3: conv path only on
device, exact residual on host.

Model (per batch element b, channel c):
  kern = leaky(d @ kW1.T) @ kW2.T            -> per-(b,c) 3x3 depthwise filter
  dw   = depthwise_conv3x3(x, kern), pad=1   (cross-correlation)
  out  = conv1x1(leaky(dw), convW) + convB + x * sigmoid-attention(d)

Sharding: data-parallel over batch B=16 across 8 cores (2 images/core),
128 SBUF partitions = (2 images x 64 channels). Device computes
  conv1x1(leaky(dw(x_fp8))) + convB          (fp16 out)
and the host adds the x * att residual in exact fp32 (att and kern are
tiny [16,64] MLPs computed on host; the fp8 diag tap weights ship
pre-built, packed IN FRONT of the x plane so one DMA chain feeds both).
PE work is 6 fp8 DoubleRow matmuls per 512 output columns:

  5 tap DRs: 9 depthwise taps as diag(kern_t) pairs at even elem deltas
  1 conv DR: (convW_hi @ act, convW_lo @ act) — both k-tiles read the SAME
             act plane via a stride-0 k-tile dim (delta=0), keeping the
             1x1 conv at ~bf16 accuracy for 0.5 PE cycles/col

i.e. 3 PE cycles/output column. Steady state per 1024-col super-chunk
(8 rows): PE 12 DRs (~1280ns), ACT two 512-col prelus (the split keeps the
prelu->conv latency off the critical loop), DVE one 1024-col psum+bias ->
fp16 staging, and one output DMA alternating between the Pool (SWDGE) and
SP (HWDGE) issue queues so neither sequencer's descriptor-gen serializes
the output stream. Warm-ups: DVE memsets
feed 3 early bf16 matmuls so the PE p-state ramp (3us, counted from the
first matmul, idle gaps don't reset it) mostly completes before the first
real DR; a tiny early Prelu hoists the ACT table load into the DMA head.
The ACT engine is the end-to-end critical chain (100% busy from the first
prelu to the last tail finish), so tail work is balanced against it.
The tail computes the last 8 rows at finer grain (4+2+2 rows) with
super-14's convs interleaved between the tap groups; tail finish ops split
across DVE and the by-then prelu-free ACT engine (Identity activation with
per-partition bias), and the last transfers fan out across the SP/Pool
queues so no sequencer or DGE path serializes the drain.
"""
import numpy as np
import ml_dtypes
import bass_rust

import concourse.bacc as bacc
import concourse.mybir as mybir
import concourse.tile as tile
from concourse.bass_utils import run_bass_kernel_spmd

F32 = mybir.dt.float32
BF16 = mybir.dt.bfloat16
FP16 = mybir.dt.float16
FP8 = mybir.dt.float8e4
AF = mybir.ActivationFunctionType
ALU = mybir.AluOpType
PM = mybir.MatmulPerfMode

B, C, H, W = 16, 64, 128, 128
NCORES = 8
BL = B // NCORES          # images per core (2)
P = BL * C                # partitions used (128)
PW = H + 2                # padded plane is PW x PW (130x130)
PLANE = PW * PW           # 16900 elements per plane
NEG = 0.1                 # leaky slope

ROWS_PER_CHUNK = 4        # output rows per conv/tap chunk (512 psum cols)
CHUNK_COLS = ROWS_PER_CHUNK * W       # 512
NSUPER = H // (2 * ROWS_PER_CHUNK)    # 16 super-chunks of 1024 cols
SUPER_COLS = 2 * CHUNK_COLS

# tap pairs: (pair index, tapA offset rel. to center, delta) — all deltas
# even (odd k-tile deltas crash the PE's DoubleRow fetcher).
TAP_PAIRS = [
    (0, -PW - 1, PW),          # t0 (-1,-1) + t3 (0,-1)
    (1, +1, PW),               # t5 (0,+1) + t8 (+1,+1)
    (2, -PW + 1, 2 * PW - 2),  # t2 (-1,+1) + t6 (+1,-1)
    (3, -PW, PW),              # t1 (-1,0) + t4 (0,0)
    (4, 0, PW),                # zero slot (@center) + t7 (+1,0): keeps every
                               # read within the chunk's guaranteed rows
]
# wpack slot s holds diag(kern[tap WDIAG_TAPS[s]]); slot 8 is zeros,
# slot 9 is t7 (see TAP_PAIRS[4]); slots 10/11 are convW hi/lo block-diag.
WDIAG_TAPS = [0, 3, 5, 8, 2, 6, 1, 4, None, 7]
NSLOT = 12
WOFF = NSLOT * P          # x plane starts after the weight pack (1536)

# x DMA row chunks of the padded plane: [r0, r1) rows, finest first so
# compute starts early. Chunk c of compute needs padded rows <= 4c+5.
XCHUNKS = [(0, 6), (6, 16), (16, 32), (32, 64), (64, 100), (100, 130)]

_CACHE = {}


def _build():
    nc = bacc.Bacc("TRN2", target_bir_lowering=False, debug=False)

    # wx = [wpack (12 x 128 fp8 diag/conv slots) | padded x plane]
    wx_d = nc.dram_tensor("wx", [P, WOFF + PLANE], FP8, kind="ExternalInput")
    out_d = nc.dram_tensor("out", [P, H * W], FP16, kind="ExternalOutput")

    with tile.TileContext(nc) as tc:
        with (
            tc.tile_pool(name="consts", bufs=1) as consts,
            tc.tile_pool(name="psA", bufs=4, space="PSUM") as psA,
            tc.tile_pool(name="psB", bufs=2, space="PSUM") as psB,
        ):
            # ---- persistent tiles ----
            # [wpack | x plane | act plane], all fp8
            xall = consts.tile([P, WOFF + 2 * PLANE], FP8)
            outst = consts.tile([P, H * W], FP16)       # output staging
            w512 = consts.tile([P, 512], BF16)          # warm-up moving data
            warm1 = consts.tile([1, 1], F32)

            xpart = list(xall.ap[0])    # [partition pitch, 128]

            # warm-ups: DVE memsets run immediately (Pool is saved for output
            # DMAs); 2 early bf16 matmuls start the PE p-state ramp clock and
            # a tiny Prelu hoists the ACT table load into the DMA head
            # the p-state ramp clock starts at the FIRST matmul, so issue a
            # tiny one as early as possible (fed by a tiny memset) before the
            # larger ramp-keeper matmuls
            wtiny = consts.tile([P, 64], BF16)
            nc.vector.memset(wtiny, 0.0)
            nc.vector.memset(w512, 0.0)
            nc.scalar.activation(warm1, wtiny[0:1, 0:1], AF.Prelu, alpha=NEG)
            wps = psB.tile([P, SUPER_COLS], F32, tag="B")
            nc.tensor.matmul(wps[0:64, 0:64], wtiny[:, 0:64], wtiny,
                             start=True, stop=True)
            for _ in range(2):
                nc.tensor.matmul(wps[:, 0:512], w512[:, 0:P], w512,
                                 start=True, stop=True)

            # input DMA ladder on SP (HWDGE): first transfer carries the
            # weight pack + the first 6 x rows in one go
            def xdma(i):
                r0, r1 = XCHUNKS[i]
                o0 = (WOFF + r0 * PW) if i else 0
                dst = bass_rust.AP(xall.tensor, o0,
                                   [xpart, [1, WOFF + r1 * PW - o0]])
                nc.sync.dma_start(out=dst,
                                  in_=wx_d.ap()[:, o0:WOFF + r1 * PW])

            xdma(0)
            xdma(1)
            for i in range(2, len(XCHUNKS)):
                xdma(i)

            def wslots(s0):             # stationary: slots [s0, s0+1]
                return bass_rust.AP(xall.tensor, s0 * P,
                                    [xpart, [P, 2], [1, P]])

            def tap_rhs(base, delta, rows):
                return bass_rust.AP(
                    xall.tensor, base,
                    [xpart, [delta, 2], [PW, rows], [1, W]])

            # ---- per-chunk pieces (cols = j0..j0+rows*W of the output) ----
            def taps(i0, rows, dst):
                base = WOFF + (i0 + 1) * PW + 1
                for (pi, off, delta) in TAP_PAIRS:
                    nc.tensor.matmul(dst, wslots(2 * pi),
                                     tap_rhs(base + off, delta, rows),
                                     start=(pi == 0), stop=(pi == 4),
                                     perf_mode=PM.DoubleRow)

            def prelu(i0, rows, pa):
                base = WOFF + PLANE + (i0 + 1) * PW + 1
                act = bass_rust.AP(xall.tensor, base,
                                   [xpart, [PW, rows], [1, W]])
                nc.scalar.activation(act, pa, AF.Prelu, alpha=NEG)

            def conv(i0, rows, dst):
                base = WOFF + PLANE + (i0 + 1) * PW + 1
                nc.tensor.matmul(dst, wslots(10), tap_rhs(base, 0, rows),
                                 start=True, stop=True,
                                 perf_mode=PM.DoubleRow)

            def fin_dve(cs, n, pb, po):
                nc.vector.tensor_copy(outst[:, cs:cs + n], pb[:, po:po + n])

            # ---- main loop over 1024-col super-chunks, software-pipelined
            # ---- by one super-chunk; the last 1024 cols run as a finer-
            # ---- grained tail ----
            prev = None     # (s, pb)
            for s in range(NSUPER - 1):
                # separate psA tiles per 512-col half: a shared tile would
                # add a WAR edge from the h0 prelu read to the h1 tap writes
                pa0 = psA.tile([P, CHUNK_COLS], F32, tag="A")
                taps(8 * s, ROWS_PER_CHUNK, pa0)
                prelu(8 * s, ROWS_PER_CHUNK, pa0)
                pa1 = psA.tile([P, CHUNK_COLS], F32, tag="A")
                taps(8 * s + 4, ROWS_PER_CHUNK, pa1)
                prelu(8 * s + 4, ROWS_PER_CHUNK, pa1)
                if prev is not None:
                    ps_, pb_ = prev
                    conv(8 * ps_, ROWS_PER_CHUNK, pb_[:, 0:CHUNK_COLS])
                    conv(8 * ps_ + 4, ROWS_PER_CHUNK, pb_[:, CHUNK_COLS:])
                    cs = SUPER_COLS * ps_
                    fin_dve(cs, SUPER_COLS, pb_, 0)
                    eng = nc.gpsimd if ps_ % 2 else nc.sync
                    eng.dma_start(
                        out=out_d.ap()[:, cs:cs + SUPER_COLS],
                        in_=outst[:, cs:cs + SUPER_COLS])
                pb = psB.tile([P, SUPER_COLS], F32, tag="B")
                prev = (s, pb)

            # tail: rows 120..128 at finer grain (4+2+2 rows). The finish
            # ops split across DVE and the (by now prelu-free) ACT engine
            # (Identity activation with per-partition bias) so neither
            # serializes the drain; tail psum tiles come from the psA pool
            # (a psB alloc would add a WAR edge to super-14's DVE read).
            s, pb_ = prev
            pt0 = psA.tile([P, CHUNK_COLS], F32, tag="A")
            taps(120, ROWS_PER_CHUNK, pt0)
            prelu(120, ROWS_PER_CHUNK, pt0)
            # super-14's convs interleave into the tail taps: each half only
            # needs its own prelu half, so they run as soon as it lands
            conv(8 * s, ROWS_PER_CHUNK, pb_[:, 0:CHUNK_COLS])
            pt1 = psA.tile([P, 256], F32, tag="A")
            taps(124, 2, pt1)
            prelu(124, 2, pt1)
            conv(8 * s + 4, ROWS_PER_CHUNK, pb_[:, CHUNK_COLS:])
            pt2 = psA.tile([P, 256], F32, tag="A")
            taps(126, 2, pt2)
            prelu(126, 2, pt2)
            cs = SUPER_COLS * s
            fin_dve(cs, SUPER_COLS, pb_, 0)
            nc.sync.dma_start(out=out_d.ap()[:, cs:cs + SUPER_COLS],
                              in_=outst[:, cs:cs + SUPER_COLS])

            def fin_act(cs2, n, pb2):
                nc.scalar.activation(outst[:, cs2:cs2 + n], pb2[:, 0:n],
                                     AF.Identity)

            pbt0 = psA.tile([P, CHUNK_COLS], F32, tag="A")
            conv(120, ROWS_PER_CHUNK, pbt0)
            fin_act(15360, 512, pbt0)
            nc.gpsimd.dma_start(out=out_d.ap()[:, 15360:15872],
                                in_=outst[:, 15360:15872])
            pbt1 = psA.tile([P, 256], F32, tag="A")
            conv(124, 2, pbt1)
            fin_dve(15872, 256, pbt1, 0)
            pbt2 = psA.tile([P, 256], F32, tag="A")
            conv(126, 2, pbt2)
            fin_act(16128, 256, pbt2)
            nc.sync.dma_start(out=out_d.ap()[:, 15872:16384],
                              in_=outst[:, 15872:16384])

    nc.compile()
    return nc


def _leaky(v):
    return np.where(v >= 0, v, NEG * v)


def kernel(x, d, kW1, kW2, convW, convB, caW1, caW2, _trace=False):
    x = np.asarray(x, np.float32)
    d = np.asarray(d, np.float32)
    kW1 = np.asarray(kW1, np.float32)
    kW2 = np.asarray(kW2, np.float32)
    convW = np.asarray(convW, np.float32)
    convB = np.asarray(convB, np.float32)
    caW1 = np.asarray(caW1, np.float32)
    caW2 = np.asarray(caW2, np.float32)
    if "nc" not in _CACHE:
        _CACHE["nc"] = _build()
    nc = _CACHE["nc"]

    E4 = ml_dtypes.float8_e4m3fn
    # tiny per-sample MLP heads on host (exact fp32)
    kern = _leaky(d @ kW1.T) @ kW2.T                       # [B, C*9]
    kern = kern.reshape(B, C, 9)
    att = 1.0 / (1.0 + np.exp(-(_leaky(d @ caW1.T) @ caW2.T)))  # [B, Cout]

    # convW block-diag (2 images) split hi+lo in fp8
    cwbd = np.zeros((P, P), np.float32)
    cwbd[0:C, 0:C] = convW.T
    cwbd[C:P, C:P] = convW.T
    cw_hi = cwbd.astype(E4)
    cw_lo = (cwbd - cw_hi.astype(np.float32)).astype(E4)

    rng = np.arange(P)
    in_maps = []
    for c in range(NCORES):
        sl = slice(c * BL, (c + 1) * BL)
        xc = x[sl].reshape(P, H, W)

        wx = np.zeros((P, WOFF + PLANE), E4)
        wpack = wx[:, :WOFF].reshape(P, NSLOT, P)
        kc = kern[sl].reshape(P, 9)                        # [(b,c), tap]
        for s, t in enumerate(WDIAG_TAPS):
            if t is None:
                continue
            wpack[rng, s, rng] = kc[:, t].astype(E4)
        wpack[:, 10, :] = cw_hi
        wpack[:, 11, :] = cw_lo
        xp = wx[:, WOFF:].reshape(P, PW, PW)
        xp[:, 1:H + 1, 1:W + 1] = xc.astype(E4)

        in_maps.append({"wx": wx})

    last_err = None
    for _attempt in range(3):
        try:
            res = run_bass_kernel_spmd(nc, in_maps,
                                       core_ids=list(range(NCORES)),
                                       trace=_trace)
            break
        except Exception as e:  # transient NRT device errors recover on retry
            last_err = e
    else:
        raise last_err

    out = np.concatenate(
        [r["out"].astype(np.float32).reshape(BL, C, H, W)
         for r in res.results], axis=0)
    out += x * att[:, :, None, None] + convB[None, :, None, None]
    if _trace:
        return out, res
    return out
